# revision 1
# baseline (speedup 1.0000x reference)
"""BEV voxel-pooling (segment_reduce) kernel for 8 Trainium2 NeuronCores.

Strategy
--------
Host (numpy, cheap — driven only by the small geometry inputs):
  * compute each point's BEV rank (bin id) exactly as the reference does
  * per sample, stable-sort points by rank; split the sorted stream into 4
    shards of ~equal point count snapped to rank boundaries (8 shards total
    across B=2 samples -> 8 cores, disjoint rank ranges)
  * per core, pack points into 128-point chunks grouped by "segment blocks"
    (128 distinct ranks per block); upload the permuted features as an
    fp16 hi/lo pair (x ~= hi + lo, error ~2^-24 — f32-class accuracy)

Device (per core, one SPMD Bass/Tile program):
  * stream feature chunks in; build a per-chunk one-hot (point -> local
    segment) on the DVE via iota/is_equal; two fp16 matmuls per chunk
    (hi and lo) accumulate segment sums into a PSUM-resident accumulator
    [128 segs x nblocks*64ch] at a per-group dynamic column offset
  * PSUM is pre-zeroed with K=1 start=True dummy matmuls (keeps all PSUM
    dependencies on the PE; walrus rejects multi-wait compute instructions)
  * copy PSUM -> SBUF once at the end (ACT) and dma_scatter_add the segment
    rows into the per-core output slice [span, 64] (output buffers are
    pre-zeroed by the runtime; scatter destinations are unique)

Host gather: place each core's [span, 64] rows into the (B, 40000, 64) grid,
reshape to the reference layout (B, C, X, Y).
"""
import sys
sys.path.insert(0, '/opt/trn_rl_repo')

import numpy as np

# ---------------- problem constants (hardcoded per spec) ----------------
B, N, C = 2, 6, 64
H_IMG, W_IMG = 256, 704
DS = 16
DSH, DSW = H_IMG // DS, W_IMG // DS          # 16, 44
D0, D1 = 4, 45                                # depth bins -> D = 41
X, Y, Z = 200, 200, 1
NBINS = X * Y * Z                             # 40000
NP_SAMPLE = N * (D1 - D0) * DSH * DSW         # 173184
NCORES = 8
SHARDS_PER_SAMPLE = 4

V = 2            # chunks per PSUM accumulation group
ABS_EVERY = 8    # absorber cadence, in groups

_compiled = {}


# ---------------- host geometry (matches reference numerics) ----------------
def _compute_ranks(frustum, post_trans, post_rots, intrinsics, extrinsics,
                   bev_res, bev_start_pos):
    frustum = np.asarray(frustum, np.float32)
    post_trans = np.asarray(post_trans, np.float32)
    post_rots = np.asarray(post_rots, np.float32)
    intrinsics = np.asarray(intrinsics, np.float32)
    extrinsics = np.asarray(extrinsics, np.float32)
    bev_res = np.asarray(bev_res, np.float32)
    bev_start_pos = np.asarray(bev_start_pos, np.float32)

    ext_inv = np.linalg.inv(extrinsics.astype(np.float64)).astype(np.float32)
    rot = ext_inv[..., :3, :3]
    trans = ext_inv[..., :3, 3]
    pts = frustum[None, None] - post_trans[:, :, None, None, None, :]
    pr_inv = np.linalg.inv(post_rots.astype(np.float64)).astype(np.float32)
    pts = np.einsum('bnij,bndhwj->bndhwi', pr_inv, pts).astype(np.float32)
    pts = np.concatenate([pts[..., :2] * pts[..., 2:3], pts[..., 2:3]], axis=-1)
    comb = (rot @ np.linalg.inv(intrinsics.astype(np.float64)).astype(np.float32)
            ).astype(np.float32)
    pts = np.einsum('bnij,bndhwj->bndhwi', comb, pts).astype(np.float32)
    geom = pts + trans[:, :, None, None, None, :]

    coords = (geom - (bev_start_pos - bev_res / 2.0)) / bev_res
    ci = coords.reshape(B, -1, 3).astype(np.int32)
    mask = ((ci[..., 0] >= 0) & (ci[..., 0] < X) &
            (ci[..., 1] >= 0) & (ci[..., 1] < Y) &
            (ci[..., 2] >= 0) & (ci[..., 2] < Z))
    rank = ci[..., 0] * (Y * Z) + ci[..., 1] * Z + ci[..., 2]
    return rank, mask


# ---------------- host planning ----------------
class CorePlan:
    __slots__ = ("order", "ranks_sorted", "lo", "seg_ranks", "nsegs",
                 "chunk_pts", "chunk_lseg", "group_block", "nchunk", "ngroups",
                 "nblocks", "span", "sample")


def _plan_cores(rank, mask, feats):
    """feats: (B, NP_SAMPLE, C) float32. Returns plans + global dims."""
    plans = []
    for b in range(B):
        r = rank[b]
        m = mask[b]
        valid_idx = np.nonzero(m)[0]
        order = valid_idx[np.argsort(r[valid_idx], kind='stable')]
        rs = r[order]
        P = len(order)
        # shard boundaries at rank changes, ~equal points
        cuts = [0]
        for s in range(1, SHARDS_PER_SAMPLE):
            i = s * P // SHARDS_PER_SAMPLE
            while i < P and rs[i] == rs[i - 1]:
                i += 1
            cuts.append(i)
        cuts.append(P)
        for s in range(SHARDS_PER_SAMPLE):
            pl = CorePlan()
            pl.sample = b
            pl.order = order[cuts[s]:cuts[s + 1]]
            pl.ranks_sorted = rs[cuts[s]:cuts[s + 1]]
            plans.append(pl)

    for pl in plans:
        rs = pl.ranks_sorted
        P = len(rs)
        # segment ids (dense, sorted)
        newseg = np.r_[True, rs[1:] != rs[:-1]]
        seg_of_pt = np.cumsum(newseg) - 1
        pl.nsegs = int(seg_of_pt[-1]) + 1 if P else 0
        pl.seg_ranks = rs[newseg]
        pl.lo = int(pl.seg_ranks[0]) if P else 0
        pl.span = int(pl.seg_ranks[-1]) - pl.lo + 1 if P else 1
        nblocks = (pl.nsegs + 127) // 128
        pl.nblocks = nblocks
        block_of_pt = seg_of_pt // 128
        # chunks per block, padded to multiple of V chunks
        chunk_pts = []     # per chunk: np.array of point indices into pl.order
        chunk_lseg = []    # per chunk: np.array [128] of local seg (255 = pad)
        group_block = []   # per group: block id
        for j in range(nblocks):
            sel = np.nonzero(block_of_pt == j)[0]
            nch = max(1, (len(sel) + 127) // 128)
            nch = ((nch + V - 1) // V) * V
            for k in range(nch):
                part = sel[k * 128:(k + 1) * 128]
                ls = np.full(128, 255, np.int32)
                ls[:len(part)] = seg_of_pt[part] - j * 128
                chunk_pts.append(part)
                chunk_lseg.append(ls)
            for g in range(nch // V):
                group_block.append(j)
        pl.chunk_pts = chunk_pts
        pl.chunk_lseg = chunk_lseg
        pl.group_block = group_block
        pl.nchunk = len(chunk_pts)
        pl.ngroups = len(group_block)

    nchunk = max(pl.nchunk for pl in plans)
    ngroups = nchunk // V
    nblocks = max(pl.nblocks for pl in plans) + 1   # +1 dummy block
    span = max(pl.span for pl in plans)
    span_pad = ((span + 127) // 128) * 128 + 1      # +1 dummy row
    return plans, nchunk, ngroups, nblocks, span_pad


def _build_inputs(pl, feats_b, nchunk, ngroups, nblocks, span_pad):
    """Per-core input arrays for the device program."""
    tok = nblocks * 128
    table = np.zeros((nchunk, 128, 2, C), np.float16)   # [c, p, hi/lo, C]
    lseg = np.full((128, nchunk), 255.0, np.float32)
    moff = np.full((1, ngroups), (nblocks - 1) * 128, np.int32)
    idx = np.full(tok, span_pad - 1, np.int16)      # default: dummy row

    for c, (part, ls) in enumerate(zip(pl.chunk_pts, pl.chunk_lseg)):
        if len(part):
            f = feats_b[pl.order[part]]             # [n, C] f32
            hi = f.astype(np.float16)
            lo = (f - hi.astype(np.float32)).astype(np.float16)
            table[c, :len(part), 0] = hi
            table[c, :len(part), 1] = lo
        lseg[:, c] = ls
    for g, j in enumerate(pl.group_block):
        moff[0, g] = j * 128
    idx[:pl.nsegs] = (pl.seg_ranks - pl.lo).astype(np.int16)
    idx_wrapped = np.tile(idx.reshape(tok // 16, 16).T, (8, 1)).copy()

    iota = np.broadcast_to(np.arange(128, dtype=np.float16), (128, 128))
    # partition-major: row p holds all chunks' (hi|lo) rows contiguously
    table_pm = np.ascontiguousarray(table.transpose(1, 0, 2, 3).reshape(128, -1))
    return {
        "table": table_pm,
        "localseg": lseg,
        "iota": np.ascontiguousarray(iota),
        "meta_off": moff,
        "scat_idx": idx_wrapped,
    }


# ---------------- device program ----------------
def _build_kernel(nchunk, ngroups, nblocks, span_pad):
    import concourse.bass as bass
    import concourse.bacc as bacc
    import concourse.mybir as mybir
    import concourse.tile as tile
    from concourse.tile_rust import add_dep_helper
    from contextlib import ExitStack

    F32 = mybir.dt.float32
    F16 = mybir.dt.float16
    I32 = mybir.dt.int32
    I16 = mybir.dt.int16
    tok = nblocks * 128

    GB = 8   # groups per feature DMA batch
    GP_TS = False  # GPSIMD tensor_scalar measured ~3x slower: keep off
    nc = bacc.Bacc()
    table = nc.dram_tensor("table", [128, nchunk * 2 * C], F16, kind="ExternalInput")
    localseg = nc.dram_tensor("localseg", [128, nchunk], F32, kind="ExternalInput")
    iota_in = nc.dram_tensor("iota", [128, 128], F16, kind="ExternalInput")
    meta_off = nc.dram_tensor("meta_off", [1, ngroups], I32, kind="ExternalInput")
    scat_idx = nc.dram_tensor("scat_idx", [128, tok // 16], I16, kind="ExternalInput")
    out = nc.dram_tensor("out", [span_pad, C], F32, kind="ExternalOutput")

    with tile.TileContext(nc) as tc, ExitStack() as ctx:
        const = ctx.enter_context(tc.tile_pool(name="const", bufs=1))
        featp = ctx.enter_context(tc.tile_pool(name="feat", bufs=3))  # 3 x 8KB/part
        ohp = ctx.enter_context(tc.tile_pool(name="oh", bufs=2 * ABS_EVERY * V))
        psump = ctx.enter_context(tc.tile_pool(name="psum", bufs=1, space="PSUM"))
        absp = ctx.enter_context(tc.tile_pool(name="abs", bufs=4))

        iota_sb = const.tile([128, 128], F16)
        nc.sync.dma_start(iota_sb[:], iota_in[:])
        lseg_sb = const.tile([128, nchunk], F32)
        nc.sync.dma_start(lseg_sb[:], localseg[:])
        moff_sb = const.tile([1, ngroups], I32)
        nc.sync.dma_start(moff_sb[:], meta_off[:])
        idx_sb = const.tile([128, tok // 16], I16)
        nc.sync.dma_start(idx_sb[:], scat_idx[:])

        psum_acc = psump.tile([128, nblocks * 128], F32)
        pdum = psump.tile([128, 64], F32, tag="pdum")

        # K=1 start=True dummy matmuls zero the accumulator (and set
        # has_written) while keeping every PSUM dependency on the PE.
        zrow = const.tile([1, 640], F16)
        nc.vector.memset(zrow[:], 0.0)
        total = nblocks * 128
        pos = 0
        while pos < total:
            n = min(512, total - pos)
            nc.tensor.matmul(psum_acc[:, pos:pos + n], zrow[:, 512:512 + 128],
                             zrow[:, 0:n], start=True, stop=True,
                             skip_group_check=True)
            pos += n

        # Pre-touch consts on DVE / Pool so TensorScalarPtr & co. never need
        # more than one cross-engine wait (walrus 1-wait limit per compute op).
        scr16 = const.tile([128, 1], F16)
        nc.vector.tensor_copy(scr16[:], iota_sb[:, 0:1])
        scr32 = const.tile([128, 1], F32)
        nc.vector.tensor_copy(scr32[:], lseg_sb[:, 0:1])
        scrg = const.tile([128, 1], I16)
        nc.gpsimd.tensor_copy(scrg[:], idx_sb[:, 0:1])
        if GP_TS:
            scr16g = const.tile([128, 1], F16)
            nc.gpsimd.tensor_copy(scr16g[:], iota_sb[:, 0:1])
            scr32g = const.tile([128, 1], F32)
            nc.gpsimd.tensor_copy(scr32g[:], lseg_sb[:, 0:1])

        absorber = None
        feat = None
        offs = None
        CW = 2 * C                      # fp16 elems per chunk per partition
        for g in range(ngroups):
            if g % GB == 0:
                nb = min(GB, ngroups - g)
                feat = featp.tile([128, GB * V * CW], F16)
                nc.sync.dma_start(
                    feat[:, :nb * V * CW],
                    table[:, g * V * CW:(g + nb) * V * CW])
                _, offs = nc.values_load_multi_w_load_instructions(
                    moff_sb[0:1, g:g + nb],
                    engines=[mybir.EngineType.PE],
                    min_val=0, max_val=(nblocks - 1) * 128,
                    skip_runtime_bounds_check=True)
            if g % ABS_EVERY == 0 and g > 0:
                # PE -> DVE progress signal through an isolated PSUM bank:
                # later one-hot builds order after it so their tile-reuse WAR
                # waits are already-observed PE ticks (elided by Tile).
                nc.tensor.matmul(pdum[:, 0:64], zrow[:, 512:512 + 128],
                                 zrow[:, 0:64], start=True, stop=True,
                                 skip_group_check=True)
                abst = absp.tile([1, 1], F32)
                absorber = nc.vector.tensor_copy(abst[:], pdum[0:1, 0:1])

            dst = psum_acc[:, bass.ds(offs[g % GB], 128)]
            for v in range(V):
                c = g * V + v
                use_gp = GP_TS and (c % 2 == 1)
                oh = ohp.tile([128, 128], F16,
                              tag="ohg" if use_gp else "oh")
                eng = nc.gpsimd if use_gp else nc.vector
                ts = eng.tensor_scalar(
                    oh[:], iota_sb[:], lseg_sb[:, c:c + 1], None,
                    mybir.AluOpType.is_equal,
                )
                if absorber is not None:
                    add_dep_helper(ts.ins, absorber.ins, sync=False,
                                   reason="order TS after PE absorber")
                base = ((g % GB) * V + v) * CW
                nc.tensor.matmul(
                    dst, oh[:], feat[:, base:base + CW],
                    start=False, stop=True, skip_group_check=True,
                )

        stage = const.tile([128, nblocks * 64], F32)
        hi_v = psum_acc[:].rearrange("p (j two c) -> p j two c", two=2, c=C)
        nc.scalar.copy(stage[:].rearrange("p (j c) -> p j c", c=C), hi_v[:, :, 0, :])
        lo_v = stage[:].rearrange("p (j c) -> p j c", c=C)
        nc.vector.tensor_add(lo_v, lo_v, hi_v[:, :, 1, :])
        nc.gpsimd.dma_scatter_add(
            out[:],
            stage[:].rearrange("p (j c) -> p j c", c=C),
            idx_sb[:],
            tok,
            tok,
            C,
            single_packet=False,
        )
    nc.finalize()
    return nc


# ---------------- entry point ----------------
def kernel(image_feature, post_trans, post_rots, intrinsics, extrinsics,
           frustum, bev_res, bev_start_pos):
    from concourse.bass_utils import run_bass_kernel_spmd
    import os

    rank, mask = _compute_ranks(frustum, post_trans, post_rots, intrinsics,
                                extrinsics, bev_res, bev_start_pos)
    feats = np.ascontiguousarray(np.asarray(image_feature, np.float32)
                                 .reshape(B, NP_SAMPLE, C))
    plans, nchunk, ngroups, nblocks, span_pad = _plan_cores(rank, mask, feats)

    in_maps = [
        _build_inputs(pl, feats[pl.sample], nchunk, ngroups, nblocks, span_pad)
        for pl in plans
    ]

    key = (nchunk, ngroups, nblocks, span_pad)
    if key not in _compiled:
        _compiled[key] = _build_kernel(*key)
    nc = _compiled[key]

    trace = bool(int(os.environ.get("BEV_TRACE", "0")))
    res = run_bass_kernel_spmd(nc, in_maps, core_ids=list(range(NCORES)),
                               trace=trace,
                               trace_cores=[0] if trace else None)
    if trace and res.exec_time_ns is not None:
        print(f"HW exec time: {res.exec_time_ns} ns")
        kernel.last_exec_time_ns = res.exec_time_ns
        kernel.last_results = res

    grid = np.zeros((B, NBINS, C), np.float32)
    for k, pl in enumerate(plans):
        o = res.results[k]["out"]
        grid[pl.sample, pl.lo:pl.lo + pl.span] = o[:pl.span]
    return np.ascontiguousarray(
        grid.reshape(B, X, Y, C).transpose(0, 3, 1, 2))



# revision 2
# speedup vs baseline: 2.3988x; 2.3988x over previous
"""BEV voxel-pooling (segment_reduce) kernel for 8 Trainium2 NeuronCores. v2

Strategy
--------
Host (numpy, layout only — no feature arithmetic):
  * compute each point's BEV rank (bin id) exactly as the reference does
  * per sample, stable-sort points by rank; split into 4 shards of ~equal
    point count snapped to rank boundaries (8 shards over B=2 -> 8 cores)
  * per core, FFD bin-pack segments (one segment = one rank) into blocks of
    <= 1024 points and <= 128 segments; every block gets a uniform budget of
    U=8 chunks of 128 points (pad chunks have local-seg 255)
  * schedule is identical across cores: NB = max block count, per-slot
    one-hot width s_i = max over cores of block-i segment count (blocks
    sorted by segment count descending so profiles align)
  * features uploaded as a single fp16 table in schedule order

Device (per core, one SPMD Bass/Tile program, all static):
  * whole feature table resident in SBUF (loaded with a few big DMAs)
  * per block: ONE batched DVE tensor_tensor(is_equal) builds all 8 chunks'
    one-hots [128 pts x s_i segs] (lseg uploaded duplicated in pairs so every
    operand's last AP dim is stride-1 x2 -> DVE 2x perf mode); the build
    also carries a dep on the feature-DMA piece so each matmul needs only
    ONE cross-engine wait (walrus 1-wait limit)
  * 8 fp16 matmuls accumulate the block into its own static PSUM slot
    [s_i x 64] (start=True zeroes it; no PSUM reuse at all)
  * ACT copies PSUM->SBUF per slot pair; per-block plain dma_start (issued
    from the otherwise-idle Pool queue) writes a window-major output
    [NB*128, 64] -- no scatter on device
  * PE is warmed up with dummy matmuls during the initial DMA so it reaches
    full clock before real work arrives

Host gather: place each block's rows at their ranks in the (B, 40000, 64)
grid (pure indexing), reshape to (B, C, X, Y).
"""
import sys
sys.path.insert(0, '/opt/trn_rl_repo')

import numpy as np

# ---------------- problem constants (hardcoded per spec) ----------------
B, N, C = 2, 6, 64
H_IMG, W_IMG = 256, 704
DS = 16
DSH, DSW = H_IMG // DS, W_IMG // DS          # 16, 44
D0, D1 = 4, 45                                # depth bins -> D = 41
X, Y, Z = 200, 200, 1
NBINS = X * Y * Z                             # 40000
NP_SAMPLE = N * (D1 - D0) * DSH * DSW         # 173184
NCORES = 8
SHARDS_PER_SAMPLE = 4

U = 8                  # chunks per block (uniform)
CAP = U * 128          # point capacity per block
SEG_LIMIT = 128        # max segments per block (PSUM partition limit)
FEAT_DMA_PIECES = 8    # big DMAs for the resident feature table
WARMUP_MM = 8          # dummy matmuls to ramp the PE clock

_compiled = {}


# ---------------- host geometry (matches reference numerics) ----------------
def _compute_ranks(frustum, post_trans, post_rots, intrinsics, extrinsics,
                   bev_res, bev_start_pos):
    frustum = np.asarray(frustum, np.float32)
    post_trans = np.asarray(post_trans, np.float32)
    post_rots = np.asarray(post_rots, np.float32)
    intrinsics = np.asarray(intrinsics, np.float32)
    extrinsics = np.asarray(extrinsics, np.float32)
    bev_res = np.asarray(bev_res, np.float32)
    bev_start_pos = np.asarray(bev_start_pos, np.float32)

    ext_inv = np.linalg.inv(extrinsics.astype(np.float64)).astype(np.float32)
    rot = ext_inv[..., :3, :3]
    trans = ext_inv[..., :3, 3]
    pts = frustum[None, None] - post_trans[:, :, None, None, None, :]
    pr_inv = np.linalg.inv(post_rots.astype(np.float64)).astype(np.float32)
    pts = np.einsum('bnij,bndhwj->bndhwi', pr_inv, pts).astype(np.float32)
    pts = np.concatenate([pts[..., :2] * pts[..., 2:3], pts[..., 2:3]], axis=-1)
    comb = (rot @ np.linalg.inv(intrinsics.astype(np.float64)).astype(np.float32)
            ).astype(np.float32)
    pts = np.einsum('bnij,bndhwj->bndhwi', comb, pts).astype(np.float32)
    geom = pts + trans[:, :, None, None, None, :]

    coords = (geom - (bev_start_pos - bev_res / 2.0)) / bev_res
    ci = coords.reshape(B, -1, 3).astype(np.int32)
    mask = ((ci[..., 0] >= 0) & (ci[..., 0] < X) &
            (ci[..., 1] >= 0) & (ci[..., 1] < Y) &
            (ci[..., 2] >= 0) & (ci[..., 2] < Z))
    rank = ci[..., 0] * (Y * Z) + ci[..., 1] * Z + ci[..., 2]
    return rank, mask


# ---------------- host planning ----------------
class CorePlan:
    __slots__ = ("sample", "order", "blocks")
    # blocks: list of (ranks[np.array], pts[np.array of indices into order])


def _plan_cores(rank, mask):
    """Shard + FFD-pack each core's segments into blocks."""
    plans = []
    for b in range(B):
        r = rank[b]
        m = mask[b]
        valid_idx = np.nonzero(m)[0]
        order = valid_idx[np.argsort(r[valid_idx], kind='stable')]
        rs = r[order]
        P = len(order)
        cuts = [0]
        for s in range(1, SHARDS_PER_SAMPLE):
            i = s * P // SHARDS_PER_SAMPLE
            while i < P and rs[i] == rs[i - 1]:
                i += 1
            cuts.append(i)
        cuts.append(P)
        for s in range(SHARDS_PER_SAMPLE):
            pl = CorePlan()
            pl.sample = b
            lo, hi = cuts[s], cuts[s + 1]
            sl_order = order[lo:hi]
            sl_rs = rs[lo:hi]
            # segments: unique ranks with start/count
            if len(sl_rs):
                newseg = np.r_[True, sl_rs[1:] != sl_rs[:-1]]
                seg_starts = np.nonzero(newseg)[0]
                seg_counts = np.diff(np.r_[seg_starts, len(sl_rs)])
                seg_ranks = sl_rs[seg_starts]
            else:
                seg_starts = np.zeros(0, np.int64)
                seg_counts = np.zeros(0, np.int64)
                seg_ranks = np.zeros(0, np.int64)
            # FFD bin packing
            desc = np.argsort(-seg_counts, kind='stable')
            bins = []   # list of [pts_total, [seg_idx,...]]
            for si in desc:
                c = int(seg_counts[si])
                placed = False
                for bn in bins:
                    if bn[0] + c <= CAP and len(bn[1]) < SEG_LIMIT:
                        bn[0] += c
                        bn[1].append(si)
                        placed = True
                        break
                if not placed:
                    bins.append([c, [si]])
            # sort bins by segment count desc so s_i profiles align
            bins.sort(key=lambda bn: -len(bn[1]))
            blocks = []
            for bn in bins:
                segs = bn[1]
                ranks = seg_ranks[segs]
                pts = np.concatenate(
                    [np.arange(seg_starts[si], seg_starts[si] + seg_counts[si])
                     for si in segs]) if segs else np.zeros(0, np.int64)
                lseg = np.concatenate(
                    [np.full(int(seg_counts[si]), j, np.int64)
                     for j, si in enumerate(segs)]) if segs else np.zeros(0, np.int64)
                blocks.append((ranks, sl_order[pts], lseg))
            pl.order = sl_order
            pl.blocks = blocks
            plans.append(pl)

    NB = max(len(pl.blocks) for pl in plans)
    if NB % 2:
        NB += 1     # pair copies need even NB
    s_prof = np.zeros(NB, np.int64)
    for pl in plans:
        for i, (ranks, _, _) in enumerate(pl.blocks):
            s_prof[i] = max(s_prof[i], len(ranks))
    s_prof = np.maximum(2, (s_prof + 1) // 2 * 2)     # even, >= 2
    return plans, NB, tuple(int(x) for x in s_prof)


def _build_inputs(pl, feats_b, NB, s_prof):
    """Per-core device input arrays (schedule order)."""
    NCH = NB * U
    table = np.zeros((128, NCH * C), np.float16)
    lseg2 = np.full((128, NCH * 2), 255.0, np.float16)
    for i, (ranks, pts, lseg) in enumerate(pl.blocks):
        n = len(pts)
        if not n:
            continue
        f = feats_b[pts].astype(np.float16)            # [n, C]
        nch = (n + 127) // 128
        fpad = np.zeros((nch * 128, C), np.float16)
        fpad[:n] = f
        lpad = np.full(nch * 128, 255, np.int64)
        lpad[:n] = lseg
        c0 = i * U
        # table[p, (c0+k)*C : ...] = fpad[k*128 + p]
        tb = fpad.reshape(nch, 128, C).transpose(1, 0, 2).reshape(128, nch * C)
        table[:, c0 * C:(c0 + nch) * C] = tb
        lv = lpad.reshape(nch, 128).T.astype(np.float16)   # [128, nch]
        lseg2[:, 2 * c0:2 * (c0 + nch)] = np.repeat(lv, 2, axis=1)
    iota = np.broadcast_to(np.arange(128, dtype=np.float16), (128, 128))
    return {
        "table": table,
        "lseg2": lseg2,
        "iota": np.ascontiguousarray(iota),
    }


# ---------------- device program ----------------
def _build_kernel(NB, s_prof):
    import concourse.bass as bass
    import concourse.bacc as bacc
    import concourse.mybir as mybir
    import concourse.tile as tile
    from concourse.tile_rust import add_dep_helper
    from contextlib import ExitStack

    F32 = mybir.dt.float32
    F16 = mybir.dt.float16
    NCH = NB * U
    sum_s = sum(s_prof)
    oh_off = np.r_[0, np.cumsum([U * s for s in s_prof])]

    nc = bacc.Bacc()
    table = nc.dram_tensor("table", [128, NCH * C], F16, kind="ExternalInput")
    lseg2_d = nc.dram_tensor("lseg2", [128, NCH * 2], F16, kind="ExternalInput")
    iota_d = nc.dram_tensor("iota", [128, 128], F16, kind="ExternalInput")
    out = nc.dram_tensor("out", [NB * 128, C], F32, kind="ExternalOutput")

    with tile.TileContext(nc) as tc, ExitStack() as ctx:
        const = ctx.enter_context(tc.tile_pool(name="const", bufs=1))

        iota_sb = const.tile([128, 128], F16)
        nc.sync.dma_start(iota_sb[:], iota_d[:])
        lseg2_sb = const.tile([128, NCH * 2], F16)
        nc.sync.dma_start(lseg2_sb[:], lseg2_d[:])

        zrow = const.tile([1, 1152], F16)
        nc.vector.memset(zrow[:], 0.0)

        feat_all = const.tile([128, NCH * C], F16)
        piece_of_chunk = np.minimum(
            (np.arange(NCH) * FEAT_DMA_PIECES) // NCH, FEAT_DMA_PIECES - 1)
        feat_dmas = []
        bnd = np.linspace(0, NCH, FEAT_DMA_PIECES + 1).astype(int)
        for pz in range(FEAT_DMA_PIECES):
            a, b_ = bnd[pz] * C, bnd[pz + 1] * C
            feat_dmas.append(
                nc.sync.dma_start(feat_all[:, a:b_], table[:, a:b_]))

        oh_all = const.tile([128, U * sum_s], F16)
        stage = const.tile([128, NB * C], F32)

        psump = ctx.enter_context(tc.tile_pool(name="psum", bufs=1, space="PSUM"))
        acc = psump.tile([128, NB * C], F32)

        # K=1 start=True matmuls zero the whole accumulator (and set
        # has_written) while doubling as the PE clock ramp during the
        # initial feature DMAs.
        total = NB * C
        pos = 0
        while pos < total:
            n = min(512, total - pos)
            nc.tensor.matmul(acc[:, pos:pos + n], zrow[:, 640:768],
                             zrow[:, 0:n], start=True, stop=True,
                             skip_group_check=True)
            pos += n

        for i in range(NB):
            s = s_prof[i]
            off = int(oh_off[i])
            # batched one-hot build for all U chunks of block i
            ov = oh_all[:, off:off + U * s].rearrange(
                "p (u j r) -> p u j r", u=U, r=2)
            i0 = (iota_sb[:, 0:s].rearrange("p (j r) -> p j r", r=2)
                  .unsqueeze(1).broadcast_to([128, U, s // 2, 2]))
            l1 = (lseg2_sb[:, 2 * U * i:2 * U * (i + 1)]
                  .rearrange("p (u r) -> p u r", r=2)
                  .unsqueeze(2).broadcast_to([128, U, s // 2, 2]))
            tt = nc.vector.tensor_tensor(ov, i0, l1, mybir.AluOpType.is_equal)
            # carry the feature-DMA dep so matmuls need only one wait
            pz = int(piece_of_chunk[i * U])
            add_dep_helper(tt.ins, feat_dmas[pz].ins, sync=True,
                           reason="gate oh-build on feat piece")
            pz_last = int(piece_of_chunk[i * U + U - 1])
            if pz_last != pz:
                add_dep_helper(tt.ins, feat_dmas[pz_last].ins, sync=True,
                               reason="gate oh-build on feat piece (end)")

            for u in range(U):
                c = i * U + u
                nc.tensor.matmul(
                    acc[0:s, i * C:(i + 1) * C],
                    oh_all[:, off + u * s:off + (u + 1) * s],
                    feat_all[:, c * C:(c + 1) * C],
                    start=False, stop=True, skip_group_check=True)

            if i % 2 == 1:
                a = (i - 1) * C
                nc.scalar.copy(stage[:, a:a + 2 * C], acc[:, a:a + 2 * C])
                nc.gpsimd.dma_start(out[(i - 1) * 128:i * 128, :],
                                    stage[:, a:a + C])
                nc.gpsimd.dma_start(out[i * 128:(i + 1) * 128, :],
                                    stage[:, a + C:a + 2 * C])
    nc.finalize()
    return nc


# ---------------- entry point ----------------
def kernel(image_feature, post_trans, post_rots, intrinsics, extrinsics,
           frustum, bev_res, bev_start_pos):
    from concourse.bass_utils import run_bass_kernel_spmd
    import os

    rank, mask = _compute_ranks(frustum, post_trans, post_rots, intrinsics,
                                extrinsics, bev_res, bev_start_pos)
    feats = np.ascontiguousarray(np.asarray(image_feature, np.float32)
                                 .reshape(B, NP_SAMPLE, C))
    plans, NB, s_prof = _plan_cores(rank, mask)

    in_maps = [_build_inputs(pl, feats[pl.sample], NB, s_prof) for pl in plans]

    key = (NB, s_prof)
    if key not in _compiled:
        _compiled[key] = _build_kernel(*key)
    nc = _compiled[key]

    trace = bool(int(os.environ.get("BEV_TRACE", "0")))
    res = run_bass_kernel_spmd(nc, in_maps, core_ids=list(range(NCORES)),
                               trace=trace,
                               trace_cores=[0] if trace else None)
    if trace and res.exec_time_ns is not None:
        print(f"HW exec time: {res.exec_time_ns} ns")
        kernel.last_exec_time_ns = res.exec_time_ns
        kernel.last_results = res

    grid = np.zeros((B, NBINS, C), np.float32)
    for k, pl in enumerate(plans):
        o = res.results[k]["out"]
        for i, (ranks, _, _) in enumerate(pl.blocks):
            n = len(ranks)
            if n:
                grid[pl.sample, ranks] = o[i * 128:i * 128 + n]
    return np.ascontiguousarray(
        grid.reshape(B, X, Y, C).transpose(0, 3, 1, 2))


# revision 3
# speedup vs baseline: 4.3691x; 1.8214x over previous
"""BEV voxel-pooling (segment_reduce) kernel for 8 Trainium2 NeuronCores. v3

Host (numpy, layout only — no feature arithmetic):
  * compute per-point BEV rank exactly as the reference does
  * per sample, stable-sort points by rank; 4 shards per sample snapped to
    rank boundaries (8 cores)
  * per core, FFD bin-pack segments (one segment = one rank) into blocks of
    <= 1024 points and <= 128 segments; blocks sorted by chunk count so the
    cross-core schedule profile (chunks U_i, one-hot width s per pair) is
    tight; all cores share one static schedule
  * features uploaded fp16 in schedule order; lseg duplicated in pairs so
    the batched one-hot build qualifies for the DVE 2x perf mode

Device (per core, one SPMD Bass/Tile program, fully static):
  * feature table resident in SBUF, streamed in pair-aligned DMA pieces
    (small first pieces so compute starts early); all on the hardware DGE
  * one batched DVE tensor_tensor(is_equal) builds a block-PAIR's one-hots;
    it also carries the dep on its feature piece so matmuls need only one
    cross-engine wait
  * per block, U_b fp16 matmuls accumulate into the block's 64-col slice of
    a per-QUAD PSUM tile (distinct tiles -> no false inter-quad deps);
    K=1 zero-matmuls pre-zero each quad and double as the PE clock ramp
  * per quad: ACT copy PSUM->SBUF (own stage tile), then one plain
    dma_start (SP queue, hardware DGE) to the window-major output
Host gather: place each block's rows at their ranks in the (B,40000,64)
grid (pure indexing), reshape to (B, C, X, Y).
"""
import sys
sys.path.insert(0, '/opt/trn_rl_repo')

import numpy as np

# ---------------- problem constants (hardcoded per spec) ----------------
B, N, C = 2, 6, 64
H_IMG, W_IMG = 256, 704
DS = 16
DSH, DSW = H_IMG // DS, W_IMG // DS          # 16, 44
D0, D1 = 4, 45                                # depth bins -> D = 41
X, Y, Z = 200, 200, 1
NBINS = X * Y * Z
NP_SAMPLE = N * (D1 - D0) * DSH * DSW         # 173184
NCORES = 8
SHARDS_PER_SAMPLE = 4

CAP = 1024             # point capacity per block
SEG_LIMIT = 128        # max segments per block (PSUM partition limit)

_compiled = {}


# ---------------- host geometry (matches reference numerics) ----------------
def _compute_ranks(frustum, post_trans, post_rots, intrinsics, extrinsics,
                   bev_res, bev_start_pos):
    frustum = np.asarray(frustum, np.float32)
    post_trans = np.asarray(post_trans, np.float32)
    post_rots = np.asarray(post_rots, np.float32)
    intrinsics = np.asarray(intrinsics, np.float32)
    extrinsics = np.asarray(extrinsics, np.float32)
    bev_res = np.asarray(bev_res, np.float32)
    bev_start_pos = np.asarray(bev_start_pos, np.float32)

    ext_inv = np.linalg.inv(extrinsics.astype(np.float64)).astype(np.float32)
    rot = ext_inv[..., :3, :3]
    trans = ext_inv[..., :3, 3]
    pts = frustum[None, None] - post_trans[:, :, None, None, None, :]
    pr_inv = np.linalg.inv(post_rots.astype(np.float64)).astype(np.float32)
    pts = np.einsum('bnij,bndhwj->bndhwi', pr_inv, pts).astype(np.float32)
    pts = np.concatenate([pts[..., :2] * pts[..., 2:3], pts[..., 2:3]], axis=-1)
    comb = (rot @ np.linalg.inv(intrinsics.astype(np.float64)).astype(np.float32)
            ).astype(np.float32)
    pts = np.einsum('bnij,bndhwj->bndhwi', comb, pts).astype(np.float32)
    geom = pts + trans[:, :, None, None, None, :]

    coords = (geom - (bev_start_pos - bev_res / 2.0)) / bev_res
    ci = coords.reshape(B, -1, 3).astype(np.int32)
    mask = ((ci[..., 0] >= 0) & (ci[..., 0] < X) &
            (ci[..., 1] >= 0) & (ci[..., 1] < Y) &
            (ci[..., 2] >= 0) & (ci[..., 2] < Z))
    rank = ci[..., 0] * (Y * Z) + ci[..., 1] * Z + ci[..., 2]
    return rank, mask


# ---------------- host planning ----------------
class CorePlan:
    __slots__ = ("sample", "blocks")
    # blocks: list of (ranks, point_indices, local_seg)


def _plan_cores(rank, mask):
    plans = []
    for b in range(B):
        r = rank[b]
        m = mask[b]
        valid_idx = np.nonzero(m)[0]
        order = valid_idx[np.argsort(r[valid_idx], kind='stable')]
        rs = r[order]
        P = len(order)
        cuts = [0]
        for s in range(1, SHARDS_PER_SAMPLE):
            i = s * P // SHARDS_PER_SAMPLE
            while i < P and rs[i] == rs[i - 1]:
                i += 1
            cuts.append(i)
        cuts.append(P)
        for s in range(SHARDS_PER_SAMPLE):
            pl = CorePlan()
            pl.sample = b
            lo, hi = cuts[s], cuts[s + 1]
            sl_order = order[lo:hi]
            sl_rs = rs[lo:hi]
            if len(sl_rs):
                newseg = np.r_[True, sl_rs[1:] != sl_rs[:-1]]
                seg_starts = np.nonzero(newseg)[0]
                seg_counts = np.diff(np.r_[seg_starts, len(sl_rs)])
                seg_ranks = sl_rs[seg_starts]
            else:
                seg_starts = seg_counts = seg_ranks = np.zeros(0, np.int64)
            desc = np.argsort(-seg_counts, kind='stable')
            bins = []
            for si in desc:
                c = int(seg_counts[si])
                placed = False
                for bn in bins:
                    if bn[0] + c <= CAP and len(bn[1]) < SEG_LIMIT:
                        bn[0] += c
                        bn[1].append(si)
                        placed = True
                        break
                if not placed:
                    bins.append([c, [si]])
            # sort by chunk count desc (primary) then seg count desc
            bins.sort(key=lambda bn: (-((bn[0] + 127) // 128), -len(bn[1])))
            blocks = []
            for bn in bins:
                segs = bn[1]
                ranks = seg_ranks[segs]
                pts = np.concatenate(
                    [np.arange(seg_starts[si], seg_starts[si] + seg_counts[si])
                     for si in segs])
                lseg = np.concatenate(
                    [np.full(int(seg_counts[si]), j, np.int64)
                     for j, si in enumerate(segs)])
                blocks.append((ranks, sl_order[pts], lseg))
            pl.blocks = blocks
            plans.append(pl)

    NB = max(len(pl.blocks) for pl in plans)
    NB += -NB % 8          # octet PSUM banks need NB % 8 == 0
    U_prof = np.ones(NB, np.int64)
    S_blk = np.zeros(NB, np.int64)
    for pl in plans:
        for i, (ranks, pts, _) in enumerate(pl.blocks):
            U_prof[i] = max(U_prof[i], (len(pts) + 127) // 128)
            S_blk[i] = max(S_blk[i], len(ranks))
    # per-PAIR one-hot width (even, >= 2)
    S_pair = []
    for i in range(NB // 2):
        s = max(int(S_blk[2 * i]), int(S_blk[2 * i + 1]), 2)
        S_pair.append(s + (s % 2))
    return plans, NB, tuple(int(u) for u in U_prof), tuple(S_pair)


def _schedule(NB, U_prof):
    """Chunk offsets per block and pair-aligned DMA piece boundaries."""
    coff = np.r_[0, np.cumsum(U_prof)]
    NCH = int(coff[-1])
    # pieces in units of PAIRS: small first, growing
    sizes = [1, 1, 2, 2, 4, 4]
    pieces = []
    p = 0
    k = 0
    while p < NB // 2:
        n = sizes[k] if k < len(sizes) else 6
        pieces.append((p, min(NB // 2, p + n)))
        p += n
        k += 1
    return coff, NCH, pieces


def _build_inputs(pl, feats_b, NB, U_prof, S_pair):
    coff, NCH, _ = _schedule(NB, U_prof)
    table = np.zeros((128, NCH * C), np.float16)
    lseg2 = np.full((128, NCH * 2), 255.0, np.float16)
    for i, (ranks, pts, lseg) in enumerate(pl.blocks):
        n = len(pts)
        if not n:
            continue
        f = feats_b[pts].astype(np.float16)
        nch = (n + 127) // 128
        fpad = np.zeros((nch * 128, C), np.float16)
        fpad[:n] = f
        lpad = np.full(nch * 128, 255, np.int64)
        lpad[:n] = lseg
        c0 = int(coff[i])
        tb = fpad.reshape(nch, 128, C).transpose(1, 0, 2).reshape(128, nch * C)
        table[:, c0 * C:(c0 + nch) * C] = tb
        lv = lpad.reshape(nch, 128).T.astype(np.float16)
        lseg2[:, 2 * c0:2 * (c0 + nch)] = np.repeat(lv, 2, axis=1)
    iota = np.zeros((128, 768), np.float16)
    iota[:, :128] = np.arange(128, dtype=np.float16)[None, :]
    return {"table": table, "lseg2": lseg2, "iota": np.ascontiguousarray(iota)}


# ---------------- device program ----------------
def _build_kernel(NB, U_prof, S_pair):
    import concourse.bass as bass
    import concourse.bacc as bacc
    import concourse.mybir as mybir
    import concourse.tile as tile
    from concourse.tile_rust import add_dep_helper
    from contextlib import ExitStack

    F32 = mybir.dt.float32
    F16 = mybir.dt.float16
    coff, NCH, pieces = _schedule(NB, U_prof)
    NBP = NB // 2
    # one-hot column offsets per pair
    oh_cols = [(int(coff[2 * i + 2] - coff[2 * i])) * S_pair[i]
               for i in range(NBP)]
    oh_off = np.r_[0, np.cumsum(oh_cols)]

    nc = bacc.Bacc()
    table = nc.dram_tensor("table", [128, NCH * C], F16, kind="ExternalInput")
    lseg2_d = nc.dram_tensor("lseg2", [128, NCH * 2], F16, kind="ExternalInput")
    iota_d = nc.dram_tensor("iota", [128, 768], F16, kind="ExternalInput")
    out = nc.dram_tensor("out", [NB * 128, C], F32, kind="ExternalOutput")

    with tile.TileContext(nc) as tc, ExitStack() as ctx:
        const = ctx.enter_context(tc.tile_pool(name="const", bufs=1))

        iota_sb = const.tile([128, 768], F16)
        nc.sync.dma_start(iota_sb[:], iota_d[:])
        zstat = iota_sb[0:1, 640:768]       # [1,128] zeros (stationary)
        zmov = iota_sb[0:1, 128:640]        # [1,512] zeros (moving)
        # (8*C = 512 == zmov width)

        lseg2_sb = const.tile([128, NCH * 2], F16)
        nc.sync.dma_start(lseg2_sb[:], lseg2_d[:])

        feat_all = const.tile([128, NCH * C], F16)
        feat_dmas = []   # per piece
        piece_of_pair = {}
        for pz, (pa, pb) in enumerate(pieces):
            a = int(coff[2 * pa]) * C
            b_ = int(coff[2 * pb]) * C
            feat_dmas.append(nc.sync.dma_start(feat_all[:, a:b_],
                                               table[:, a:b_]))
            for i in range(pa, pb):
                piece_of_pair[i] = pz

        oh_all = const.tile([128, int(oh_off[-1])], F16)

        psump = ctx.enter_context(
            tc.tile_pool(name="psum", bufs=1, space="PSUM"))
        quads = [psump.tile([128, 8 * C], F32, name=f"quad{j}", tag=f"q{j}")
                 for j in range(NB // 8)]
        stages = [const.tile([128, 8 * C], F32, name=f"stage{j}")
                  for j in range(NB // 8)]

        # zero each accumulator bank (K=1 matmuls) — doubles as PE ramp
        for j in range(NB // 8):
            nc.tensor.matmul(quads[j][:, 0:8 * C], zstat, zmov[:, 0:8 * C],
                             start=True, stop=True, skip_group_check=True)

        for i in range(NBP):
            s = S_pair[i]
            off = int(oh_off[i])
            cnt = int(coff[2 * i + 2] - coff[2 * i])    # chunks in pair
            c0 = int(coff[2 * i])
            ov = oh_all[:, off:off + cnt * s].rearrange(
                "p (u j r) -> p u j r", u=cnt, r=2)
            i0 = (iota_sb[:, 0:s].rearrange("p (j r) -> p j r", r=2)
                  .unsqueeze(1).broadcast_to([128, cnt, s // 2, 2]))
            l1 = (lseg2_sb[:, 2 * c0:2 * (c0 + cnt)]
                  .rearrange("p (u r) -> p u r", r=2)
                  .unsqueeze(2).broadcast_to([128, cnt, s // 2, 2]))
            tt = nc.vector.tensor_tensor(ov, i0, l1, mybir.AluOpType.is_equal)
            add_dep_helper(tt.ins, feat_dmas[piece_of_pair[i]].ins, sync=True,
                           reason="gate oh-build on feat piece")

            qt = quads[i // 4]
            for half in range(2):
                b_ = 2 * i + half
                ub = int(U_prof[b_])
                col = (b_ % 8) * C
                for u in range(ub):
                    c = int(coff[b_]) + u
                    nc.tensor.matmul(
                        qt[0:s, col:col + C],
                        oh_all[:, off + (c - c0) * s:off + (c - c0 + 1) * s],
                        feat_all[:, c * C:(c + 1) * C],
                        start=False, stop=True, skip_group_check=True)

            if i % 4 == 3:
                j = i // 4
                nc.scalar.copy(stages[j][:], quads[j][:])
                nc.sync.dma_start(
                    out[j * 1024:(j + 1) * 1024, :].rearrange(
                        "(k p) c -> p k c", k=8),
                    stages[j][:].rearrange("p (k c) -> p k c", c=C))
    nc.finalize()
    return nc


# ---------------- entry point ----------------
def kernel(image_feature, post_trans, post_rots, intrinsics, extrinsics,
           frustum, bev_res, bev_start_pos):
    from concourse.bass_utils import run_bass_kernel_spmd
    import os

    rank, mask = _compute_ranks(frustum, post_trans, post_rots, intrinsics,
                                extrinsics, bev_res, bev_start_pos)
    feats = np.ascontiguousarray(np.asarray(image_feature, np.float32)
                                 .reshape(B, NP_SAMPLE, C))
    plans, NB, U_prof, S_pair = _plan_cores(rank, mask)

    in_maps = [_build_inputs(pl, feats[pl.sample], NB, U_prof, S_pair)
               for pl in plans]

    key = (NB, U_prof, S_pair)
    if key not in _compiled:
        _compiled[key] = _build_kernel(*key)
    nc = _compiled[key]

    trace = bool(int(os.environ.get("BEV_TRACE", "0")))
    res = run_bass_kernel_spmd(nc, in_maps, core_ids=list(range(NCORES)),
                               trace=trace,
                               trace_cores=[0] if trace else None)
    if trace and res.exec_time_ns is not None:
        print(f"HW exec time: {res.exec_time_ns} ns")
        kernel.last_exec_time_ns = res.exec_time_ns
        kernel.last_results = res

    grid = np.zeros((B, NBINS, C), np.float32)
    for k, pl in enumerate(plans):
        o = res.results[k]["out"]
        for i, (ranks, _, _) in enumerate(pl.blocks):
            n = len(ranks)
            if n:
                grid[pl.sample, ranks] = o[i * 128:i * 128 + n]
    return np.ascontiguousarray(
        grid.reshape(B, X, Y, C).transpose(0, 3, 1, 2))
